# revision 1
# baseline (speedup 1.0000x reference)
"""GPT causal attention (B=2,S=4096,D=768,H=12,HD=64) on 8 NeuronCores.

Sharding: core c handles batch b=c//4 and head-group hg=c%4 (3 heads each).
Per-core kernel (bf16 matmuls, fp32 PSUM):
  - scores computed TRANSPOSED per 128-key chunk into a 3-head fused PSUM
    tile [128, 3, 512]; ONE exp activation per (qsb, kc) covers all 3 heads
  - P@V reoriented: es is the STATIONARY operand (stationary loads are free),
    V [128,65] the moving one -> 65 cols/block instead of 128; col 64 of V is
    ones so ctx PSUM col 64 accumulates the softmax denominator
  - normalization: DVE reciprocal of the denominator column + per-partition
    tensor_scalar_mul while draining ctx PSUM -> SBUF bf16
  - ctx transposed to head-dim-major via PE pair-transposes (identity
    matmul) staged through scratch-bank bf16 slots
  - PSUM: scores double-buffer 6 banks + ctx accumulators 2 banks. The QKV /
    output projections run in a pre-PV window at each qsb start, REUSING the
    ctx banks as scratch (P@V for the first chunks is deferred; their es
    tiles are buffered). The scores PSUM rotation is never perturbed, so the
    Tensor engine stays a chunk ahead of Act (exp), which is the per-chunk
    bottleneck.
  - output written transposed (outT [768, 4096] f32); host sums the 4
    head-group partials per batch and adds bo.
"""
import contextlib
import sys

sys.path.insert(0, "/opt/trn_rl_repo")

import numpy as np
import ml_dtypes

import concourse.bass as bass
import concourse.tile as tile
from concourse import bacc, mybir
from concourse.bass_utils import run_bass_kernel_spmd

B, S, D, H, HD = 2, 4096, 768, 12, 64
N_CORES = 8
HPC = 3           # heads per core
DH = HPC * HD     # 192 per-core head dims
KD = D // 128     # 6 contraction chunks
QSB = 512         # query superblock width
NQSB = S // QSB   # 8
NKC = S // 128    # 32 key chunks

f32 = mybir.dt.float32
bf16 = mybir.dt.bfloat16
BF = ml_dtypes.bfloat16
EXP = mybir.ActivationFunctionType.Exp

_CACHE = {}


def build():
    nc = bacc.Bacc("TRN2", target_bir_lowering=False, debug=False,
                   num_devices=N_CORES)
    xT = nc.dram_tensor("xT", [D, S], bf16, kind="ExternalInput").ap()
    wqk = nc.dram_tensor("wqk", [D, HPC, 128], bf16, kind="ExternalInput").ap()
    wv = nc.dram_tensor("wv", [D, DH], bf16, kind="ExternalInput").ap()
    woT = nc.dram_tensor("woT", [DH, D], bf16, kind="ExternalInput").ap()
    qkb = nc.dram_tensor("qkb", [128, HPC], f32, kind="ExternalInput").ap()
    bvv = nc.dram_tensor("bvv", [1, DH], bf16, kind="ExternalInput").ap()
    tmk = nc.dram_tensor("tmk", [128, 128], bf16, kind="ExternalInput").ap()
    idm = nc.dram_tensor("idm", [128, 128], bf16, kind="ExternalInput").ap()
    outT = nc.dram_tensor("outT", [D, S], f32, kind="ExternalOutput").ap()

    with tile.TileContext(nc) as tc:
        with contextlib.ExitStack() as ctx:
            sb = ctx.enter_context(tc.tile_pool(name="sb", bufs=1))
            # ---- resident inputs ----
            wqk_sb = sb.tile([128, KD, HPC, 128], bf16, tag="wqk", name="wqk_sb")
            nc.sync.dma_start(
                out=wqk_sb, in_=wqk.rearrange("(ko p) h m -> p ko h m", p=128))
            wv_sb = sb.tile([128, KD, DH], bf16, tag="wv", name="wv_sb")
            nc.sync.dma_start(
                out=wv_sb, in_=wv.rearrange("(ko p) m -> p ko m", p=128))
            qkb_sb = sb.tile([128, HPC], f32, tag="qkb", name="qkb_sb")
            nc.sync.dma_start(out=qkb_sb, in_=qkb)
            bv_sb = sb.tile([1, DH], bf16, tag="bv", name="bv_sb")
            nc.sync.dma_start(out=bv_sb, in_=bvv)
            mask_sb = sb.tile([128, 128], bf16, tag="mk", name="mask_sb")
            nc.sync.dma_start(out=mask_sb, in_=tmk)
            id_sb = sb.tile([128, 128], bf16, tag="id", name="id_sb")
            nc.sync.dma_start(out=id_sb, in_=idm)
            xt = [sb.tile([128, S], bf16, tag=f"xt{k}", name=f"xt{k}")
                  for k in range(KD)]
            for k in range(KD):
                nc.sync.dma_start(out=xt[k][:, 0:QSB],
                                  in_=xT[k * 128:(k + 1) * 128, 0:QSB])
            wo_a = sb.tile([128, D], bf16, tag="woa", name="wo_a")
            # head-2 rows of woT duplicated into BOTH partition halves, so
            # either half of a pair-transposed cnb tile can contract with it
            wo_b2 = sb.tile([128, D], bf16, tag="wob", name="wo_b2")
            nc.sync.dma_start(out=wo_a, in_=woT[0:128, :])
            nc.sync.dma_start(out=wo_b2[0:64, :], in_=woT[128:DH, :])
            nc.sync.dma_start(out=wo_b2[64:128, :], in_=woT[128:DH, :])
            ones128 = sb.tile([1, 128], bf16, tag="o1", name="ones128")
            nc.vector.memset(ones128, 1.0)
            outTr = outT.rearrange("(o p) s -> p o s", p=128)

            # mask broadcast AP over the 3 heads (stride-0 middle dim)
            m_ap = mask_sb[:, :]
            mask_b = bass.AP(tensor=m_ap.tensor, offset=m_ap.offset,
                             ap=[list(m_ap.ap[0]), [0, HPC], list(m_ap.ap[1])])

            # ---- qkv storage ----
            # QT_a/KT_a: heads 0,1 stacked on partitions; QKb: head2 Q (top)
            # + head2 K (bottom); KB2: head2 K repartitioned to base 0 via DMA
            QT_a = sb.tile([128, S], bf16, tag="qta", name="QT_a")
            KT_a = sb.tile([128, S], bf16, tag="kta", name="KT_a")
            QKb = sb.tile([128, S], bf16, tag="qkb2", name="QKb")
            KB2 = sb.tile([64, S], bf16, tag="kb2", name="KB2")
            V_sb = sb.tile([128, NKC, HPC, HD + 1], bf16, tag="vsb", name="V_sb")
            nc.vector.memset(V_sb[:, :, :, HD:HD + 1], 1.0)

            sps = ctx.enter_context(tc.tile_pool(name="sps", bufs=2, space="PSUM"))
            cps = ctx.enter_context(tc.tile_pool(name="cps", bufs=1, space="PSUM"))
            att = ctx.enter_context(tc.tile_pool(name="att", bufs=18))
            nrm = ctx.enter_context(tc.tile_pool(name="nrm", bufs=2))
            stg = ctx.enter_context(tc.tile_pool(name="stg", bufs=2))

            def grab_cab():
                # ctx banks, either PV accumulators or projection scratch
                cA = cps.tile([128, 7 * (HD + 1)], f32, tag="cA", name="cA")
                cB = cps.tile([128, 5 * (HD + 1)], f32, tag="cB", name="cB")
                return cA, cB

            def emit_qk_pass(n, i):
                # pass i: 0 -> QT_a (Q h0|h1), 1 -> KT_a (K h0|h1),
                #         2 -> QKb (Q h2 | K h2) + repartition K h2 -> KB2
                cA, cB = grab_cab()
                cols = slice(n * QSB, (n + 1) * QSB)
                c0 = slice(n * QSB, n * QSB + 448)
                c1 = slice(n * QSB + 448, (n + 1) * QSB)
                for k in range(KD):
                    nc.tensor.matmul(cA[:, 0:448], wqk_sb[:, k, i, :],
                                     xt[k][:, c0], start=(k == 0),
                                     stop=(k == KD - 1))
                    nc.tensor.matmul(cB[:, 0:64], wqk_sb[:, k, i, :],
                                     xt[k][:, c1], start=(k == 0),
                                     stop=(k == KD - 1))
                dst = (QT_a, KT_a, QKb)[i]
                nc.vector.tensor_scalar_add(dst[:, c0], cA[:, 0:448],
                                            qkb_sb[:, i:i + 1])
                nc.vector.tensor_scalar_add(dst[:, c1], cB[:, 0:64],
                                            qkb_sb[:, i:i + 1])
                if i == 2:
                    nc.sync.dma_start(out=KB2[:, cols], in_=QKb[64:128, cols])

            def emit_v_pair(n, jp):
                # V for two 128-token chunks (pair jp) of token chunk n.
                # PSUM zero regions are 2KB banks: only the first matmul in
                # the bank uses start=True; the second chunk's first write
                # lands on pending-zero bytes (overwrite), then accumulates.
                cA, _ = grab_cab()
                for j in (2 * jp, 2 * jp + 1):
                    t = 4 * n + j
                    po = cA[:, (j % 2) * 192:(j % 2) * 192 + DH]
                    tcols = slice(t * 128, (t + 1) * 128)
                    for k in range(KD):
                        nc.tensor.matmul(po, xt[k][:, tcols], wv_sb[:, k, :],
                                         start=(k == 0 and j % 2 == 0),
                                         stop=False, skip_group_check=True)
                    nc.tensor.matmul(po, ones128, bv_sb, start=False,
                                     stop=(j % 2 == 1),
                                     skip_group_check=True)
                    nc.vector.tensor_copy(
                        V_sb[:, t, :, 0:HD],
                        po.rearrange("p (h d) -> p h d", h=HPC))

            def emit_outproj_oc(m, cnaq, cnb2, i, osb):
                # output projection for qsb m, dout chunk i.
                # cnaq[:, qs, :]: heads 0|1 hd on partitions, 128 queries;
                # cnb2[64*(qs%2):, qs//2, :]: head 2 hd, same queries.
                cA, cB = grab_cab()
                oc = slice(i * 128, (i + 1) * 128)
                for qs in range(4):
                    po = (cA[:, qs * 128:(qs + 1) * 128] if qs < 3
                          else cB[:, 0:128])
                    b0 = 64 * (qs % 2)
                    nc.tensor.matmul(po, wo_a[:, oc], cnaq[:, qs, :],
                                     start=(qs in (0, 3)), stop=False,
                                     skip_group_check=True)
                    nc.tensor.matmul(po, wo_b2[b0:b0 + 64, oc],
                                     cnb2[b0:b0 + 64, qs // 2, :],
                                     start=False, stop=(qs in (2, 3)),
                                     skip_group_check=True)
                nc.vector.tensor_copy(osb[:, i, 0:384], cA[:, 0:384])
                nc.vector.tensor_copy(osb[:, i, 384:512], cB[:, 0:128])
                if i == 5:
                    # one batched store per qsb: dram side strided over the
                    # six 128-row dout blocks
                    nc.sync.dma_start(
                        out=outTr[:, :, m * QSB:(m + 1) * QSB], in_=osb)

            def emit_transposes(pA, pB, cnaq, cnb2):
                # PE-transpose the previous qsb's normalized ctx into
                # head-dim-major tiles: one 128-wide transpose per head-pair
                # (in [128q, 2x64hd] -> out [2x64hd, 128q]), staged through
                # bf16 slots in the scratch bank
                cAt, _ = grab_cab()
                srcs = [pA[:, qs, :, :] for qs in range(4)]
                srcs += [pB[:, 2 * p:2 * p + 2, :] for p in range(2)]
                dsts = [cnaq[:, qs, :] for qs in range(4)]
                dsts += [cnb2[:, p, :] for p in range(2)]
                for i, (src, dst) in enumerate(zip(srcs, dsts)):
                    out = cAt[:, i * 64:(i + 1) * 64].bitcast(bf16)
                    nc.tensor.matmul(out, src, id_sb[:, :],
                                     is_transpose=True, start=(i == 0),
                                     stop=(i == 5), skip_group_check=True)
                    nc.vector.tensor_copy(dst, out)

            keep = {}
            keep_cn = {}
            outproj_sched = {4: [0], 5: [1, 2], 6: [3, 4], 7: [5, 6]}

            # projections for token chunks 0 and 1 run before the first
            # qsb (overlapping the input DMA trickle)
            emit_qk_pass(0, 0)
            emit_qk_pass(0, 2)
            emit_qk_pass(0, 1)
            emit_v_pair(0, 0)
            emit_v_pair(0, 1)
            # bulk x tails load behind the latency-critical startup DMAs
            for k in range(KD):
                nc.sync.dma_start(out=xt[k][:, QSB:S],
                                  in_=xT[k * 128:(k + 1) * 128, QSB:S])
            emit_qk_pass(1, 0)
            emit_qk_pass(1, 2)
            emit_qk_pass(1, 1)
            emit_v_pair(1, 0)
            emit_v_pair(1, 1)

            for n in range(NQSB):
                nkc = 4 * n + 4
                q0 = n * QSB
                # chunks whose P@V (and drains) are deferred past the scratch
                # window; their es tiles stay buffered so the scores->exp
                # stream keeps Act busy while PE does projection work in the
                # ctx scratch banks
                pre = min(nkc, 16) if n <= 3 else min(2 * n, 14)

                ctxnA = nrm.tile([128, 4, 2, HD], bf16, tag="ctxnA",
                                 name="ctxnA")
                ctxnB = nrm.tile([128, 4, HD], bf16, tag="ctxnB", name="ctxnB")
                rcp = nrm.tile([128, 4 * HPC], f32, tag="rcp", name="rcp")

                def emit_scores(kc):
                    off = max(0, kc - 4 * n) * 128
                    sp = sps.tile([128, HPC, QSB], f32, tag="sp", name="sp")
                    kk = slice(kc * 128, (kc + 1) * 128)
                    qq = slice(q0 + off, q0 + QSB)
                    nc.tensor.matmul(sp[:, 0, off:], KT_a[0:64, kk],
                                     QT_a[0:64, qq], start=True, stop=True)
                    nc.tensor.matmul(sp[:, 1, off:], KT_a[64:128, kk],
                                     QT_a[64:128, qq], start=True, stop=True)
                    nc.tensor.matmul(sp[:, 2, off:], KB2[:, kk],
                                     QKb[0:64, qq], start=True, stop=True)
                    es = att.tile([128, HPC, QSB], bf16, tag="es", name="es")
                    nc.scalar.activation(es[:, :, off:], sp[:, :, off:], EXP,
                                         scale=0.125)
                    if kc >= 4 * n:
                        nc.vector.tensor_mul(es[:, :, off:off + 128],
                                             es[:, :, off:off + 128], mask_b)
                    return es

                def emit_pv(kc, es, cA, cB, started):
                    # one start=True per 2KB bank per qsb: the other slots'
                    # first writes land on pending-zero bytes (overwrite),
                    # then accumulate across kc
                    off = max(0, kc - 4 * n) * 128
                    for h in range(HPC):
                        for qs in range(off // 128, 4):
                            g = h * 4 + qs
                            bank = int(g >= 7)
                            sl = (cA[:, g * 65:g * 65 + 65] if g < 7
                                  else cB[:, (g - 7) * 65:(g - 7) * 65 + 65])
                            st = kc == 0 and bank not in started
                            if st:
                                started.add(bank)
                            nc.tensor.matmul(
                                sl, es[:, h, qs * 128:(qs + 1) * 128],
                                V_sb[:, kc, h, :],
                                start=st, stop=(kc == 4 * n + qs),
                                skip_group_check=True)
                    # drain the qsub that just completed its column
                    if kc >= 4 * n:
                        j = kc - 4 * n
                        for h in range(HPC):
                            g = h * 4 + j
                            sl = (cA[:, g * 65:g * 65 + 65] if g < 7
                                  else cB[:, (g - 7) * 65:(g - 7) * 65 + 65])
                            dst = (ctxnA[:, j, h, :] if h < 2
                                   else ctxnB[:, j, :])
                            nc.vector.reciprocal(rcp[:, g:g + 1],
                                                 sl[:, HD:HD + 1])
                            nc.vector.tensor_scalar_mul(dst, sl[:, 0:HD],
                                                        rcp[:, g:g + 1])

                # scratch-window work: transposes + output projection for
                # qsb n-1, projection for chunk n+1 -- all in the ctx
                # scratch banks
                work = []
                if n > 0:
                    cnaq = nrm.tile([128, 4, 128], bf16, tag="cna", bufs=8,
                                    name="cnaq")
                    cnb2 = nrm.tile([128, 2, 128], bf16, tag="cnb", bufs=8,
                                    name="cnb2")
                    keep_cn[n - 1] = (cnaq, cnb2)
                    pA, pB = keep[n - 1]
                    work.append(lambda a=cnaq, b=cnb2, pa=pA, pb=pB:
                                emit_transposes(pa, pb, a, b))
                if n + 2 < NQSB:
                    work.append(lambda m=n + 2: emit_qk_pass(m, 0))
                    work.append(lambda m=n + 2: emit_qk_pass(m, 2))
                    work.append(lambda m=n + 2: emit_qk_pass(m, 1))
                    work.append(lambda m=n + 2: emit_v_pair(m, 0))
                    work.append(lambda m=n + 2: emit_v_pair(m, 1))
                # output projections deferred to the slack-rich late windows
                for m in outproj_sched.get(n, ()):
                    osb = stg.tile([128, 6, QSB], f32, tag="osb", name="osb")
                    a, b = keep_cn[m]
                    for i in range(6):
                        work.append(
                            lambda a=a, b=b, i=i, m=m, o=osb:
                            emit_outproj_oc(m, a, b, i, o))

                # ---- pre-PV window: scores stream + scratch work ----
                es_q = []
                for kc in range(pre):
                    es_q.append(emit_scores(kc))
                    left = pre - kc - 1
                    npop = (len(work) if left == 0
                            else -(-len(work) // (left + 1)))
                    for _ in range(min(npop, len(work))):
                        work.pop(0)()
                while work:
                    work.pop(0)()
                # ---- P@V catch-up, then steady skewed loop ----
                # small qsbs: bulk catch-up; large qsbs: drain the deferred
                # P@V backlog one chunk per steady iteration (PSUM adds are
                # commutative; each slot's drain still comes last)
                cA, cB = grab_cab()
                started = set()
                backlog = list(range(pre))
                if n <= 3:
                    for kc in backlog:
                        emit_pv(kc, es_q[kc], cA, cB, started)
                    backlog = []
                for kc in range(pre, nkc):
                    es_q.append(emit_scores(kc))
                    if backlog:
                        j = backlog.pop(0)
                        emit_pv(j, es_q[j], cA, cB, started)
                    if kc > pre:
                        emit_pv(kc - 1, es_q[kc - 1], cA, cB, started)
                for j in backlog:
                    emit_pv(j, es_q[j], cA, cB, started)
                if nkc > pre:
                    emit_pv(nkc - 1, es_q[nkc - 1], cA, cB, started)
                keep[n] = (ctxnA, ctxnB)
            # tail: transposes + output projection of the last qsb
            cnaq = nrm.tile([128, 4, 128], bf16, tag="cna", bufs=8,
                            name="cnaq")
            cnb2 = nrm.tile([128, 2, 128], bf16, tag="cnb", bufs=8,
                            name="cnb2")
            emit_transposes(*keep[NQSB - 1], cnaq, cnb2)
            osb = stg.tile([128, 6, QSB], f32, tag="osb", name="osb")
            for i in range(6):
                emit_outproj_oc(NQSB - 1, cnaq, cnb2, i, osb)
    nc.compile()
    return nc


def _in_maps(x, Wq, bq, Wk, bk, Wv, bv, Wo, bo):
    tri = np.triu(np.ones((128, 128), np.float32)).astype(BF)
    WqT, WkT, WvT = Wq.T, Wk.T, Wv.T
    maps = []
    for c in range(N_CORES):
        b, hg = c // 4, c % 4
        sl = slice(DH * hg, DH * hg + DH)
        wqk = np.empty((D, HPC, 128), np.float32)
        qkb = np.zeros((128, HPC), np.float32)
        h0, h1, h2 = (DH * hg + HD * h for h in range(HPC))
        # pass 0: Q heads 0|1; pass 1: K heads 0|1; pass 2: Q h2 | K h2
        wqk[:, 0, 0:64] = WqT[:, h0:h0 + HD]
        wqk[:, 0, 64:128] = WqT[:, h1:h1 + HD]
        wqk[:, 1, 0:64] = WkT[:, h0:h0 + HD]
        wqk[:, 1, 64:128] = WkT[:, h1:h1 + HD]
        wqk[:, 2, 0:64] = WqT[:, h2:h2 + HD]
        wqk[:, 2, 64:128] = WkT[:, h2:h2 + HD]
        qkb[0:64, 0] = bq[h0:h0 + HD]
        qkb[64:128, 0] = bq[h1:h1 + HD]
        qkb[0:64, 1] = bk[h0:h0 + HD]
        qkb[64:128, 1] = bk[h1:h1 + HD]
        qkb[0:64, 2] = bq[h2:h2 + HD]
        qkb[64:128, 2] = bk[h2:h2 + HD]
        maps.append({
            "xT": np.ascontiguousarray(x[b].T).astype(BF),
            "wqk": wqk.astype(BF),
            "wv": np.ascontiguousarray(WvT[:, sl]).astype(BF),
            "woT": np.ascontiguousarray(Wo[:, sl].T).astype(BF),
            "qkb": qkb,
            "bvv": bv[sl].reshape(1, DH).astype(BF),
            "tmk": tri,
            "idm": np.eye(128, dtype=np.float32).astype(BF),
        })
    return maps


def kernel(x, Wq, bq, Wk, bk, Wv, bv, Wo, bo):
    if "nc" not in _CACHE:
        _CACHE["nc"] = build()
    nc = _CACHE["nc"]
    maps = _in_maps(x, Wq, bq, Wk, bk, Wv, bv, Wo, bo)
    res = run_bass_kernel_spmd(nc, maps, list(range(N_CORES))).results
    out = np.zeros((B, S, D), np.float32)
    for c in range(N_CORES):
        out[c // 4] += res[c]["outT"].T
    out += bo.astype(np.float32)
    return out

